# revision 7
# baseline (speedup 1.0000x reference)
"""Trainium2 Bass kernel for 2D-relative-bias multi-head attention.

Shapes (hardcoded): x [64, 16, 16, 512], 16 heads x 32 dim, S = 256.
Sharding: data-parallel over batch, 8 batches per core on 8 cores.

Per-core device pipeline (all matmuls bf16, fp32 PSUM accumulation):
  qT/kT = W^T @ x^T            [nd, tok]   (PE, K=c)
  v     = x @ Wv               [tok, nd]   (PE)
  logitsT[t,s] per head        (PE, K=32, 4-head row-packed via tile_position)
  E0 = exp(logitsT)            (ACT, PSUM->SBUF bf16)
  E  = E0 * exp(biasT)         (DVE, bias table precomputed on host)
  sums = 1^T E (replicated)    (PE, 4-head col-packed, all-ones lhsT)
  out_unT = V^T E              (PE, 4-head col-packed)
  R = 1/sums                   (DVE reciprocal_approx_fast)
  outT = out_unT * R           (DVE)
  final = outT^T @ Wo + o_b    (PE)
"""

import numpy as np
import ml_dtypes

try:
    import concourse.bass as bass
except ImportError:  # pragma: no cover
    import sys

    sys.path.insert(0, "/opt/trn_rl_repo")
    import concourse.bass as bass
from concourse import bacc

import concourse.mybir as mybir
import concourse.tile as tile
from concourse.bass_utils import run_bass_kernel_spmd

BF16 = mybir.dt.bfloat16
F32 = mybir.dt.float32
AF = mybir.ActivationFunctionType
OP = mybir.AluOpType

B, H, W, C = 64, 16, 16, 512
NH, D = 16, 32
S = H * W            # 256
NCORES = 8
BPC = B // NCORES    # 8 batches per core
TOK = BPC * S        # 2048 tokens per core
SCALE = D ** -0.5


def build_program(reps: int = 1, debug: bool = False):
    nc = bacc.Bacc()
    xT_d = nc.dram_tensor("xT", [4, 128, TOK], BF16, kind="ExternalInput")
    wq_d = nc.dram_tensor("wq", [4, 128, 512], BF16, kind="ExternalInput")
    wk_d = nc.dram_tensor("wk", [4, 128, 512], BF16, kind="ExternalInput")
    wv_d = nc.dram_tensor("wv", [4, 128, 512], BF16, kind="ExternalInput")
    wo_d = nc.dram_tensor("wo", [4, 128, 512], BF16, kind="ExternalInput")
    expb_d = nc.dram_tensor("expb", [2, 128, NH * S], BF16, kind="ExternalInput")
    qb_d = nc.dram_tensor("qb", [1, 512], BF16, kind="ExternalInput")
    kb_d = nc.dram_tensor("kb", [1, 512], BF16, kind="ExternalInput")
    vb_d = nc.dram_tensor("vb", [1, 512], BF16, kind="ExternalInput")
    ob_d = nc.dram_tensor("ob", [1, 512], BF16, kind="ExternalInput")
    ones_r_d = nc.dram_tensor("ones_r", [1, 512], BF16, kind="ExternalInput")
    ones_c_d = nc.dram_tensor("ones_c", [128, 32], BF16, kind="ExternalInput")
    out_d = nc.dram_tensor("out", [TOK, 512], F32, kind="ExternalOutput")
    if debug:
        dbg_qT = nc.dram_tensor("dbg_qT", [128, TOK], F32, kind="ExternalOutput")
        dbg_kT = nc.dram_tensor("dbg_kT", [128, TOK], F32, kind="ExternalOutput")
        dbg_v = nc.dram_tensor("dbg_v", [128, 512], F32, kind="ExternalOutput")
        dbg_e0 = nc.dram_tensor("dbg_e0", [128, NH * S], F32, kind="ExternalOutput")
        dbg_e = nc.dram_tensor("dbg_e", [128, NH * S], F32, kind="ExternalOutput")
        dbg_s = nc.dram_tensor("dbg_s", [128, 1024], F32, kind="ExternalOutput")
        dbg_r = nc.dram_tensor("dbg_r", [128, 1024], F32, kind="ExternalOutput")
        dbg_ot = nc.dram_tensor("dbg_ot", [128, 256], F32, kind="ExternalOutput")

    with tile.TileContext(nc) as tc:
        import contextlib

        with contextlib.ExitStack() as ctx:
            wpool = ctx.enter_context(tc.tile_pool(name="wpool", bufs=1))
            xpool = ctx.enter_context(tc.tile_pool(name="xpool", bufs=1))
            qkpool = ctx.enter_context(tc.tile_pool(name="qkpool", bufs=1))
            epool = ctx.enter_context(tc.tile_pool(name="epool", bufs=2))
            rpool = ctx.enter_context(tc.tile_pool(name="rpool", bufs=2))
            otpool = ctx.enter_context(tc.tile_pool(name="otpool", bufs=8))
            fpool = ctx.enter_context(tc.tile_pool(name="fpool", bufs=3))
            dpool = ctx.enter_context(tc.tile_pool(name="dpool", bufs=1)) if debug else None
            pl_pool = ctx.enter_context(
                tc.tile_pool(name="pl", bufs=1, space="PSUM"))
            pa_pool = ctx.enter_context(
                tc.tile_pool(name="pa", bufs=2, space="PSUM"))
            ps_pool = ctx.enter_context(
                tc.tile_pool(name="ps", bufs=1, space="PSUM"))

            # ---- persistent constants ----
            wq = [wpool.tile([128, 512], BF16, name=f"wq{i}", tag=f"wq{i}") for i in range(4)]
            wk = [wpool.tile([128, 512], BF16, name=f"wk{i}", tag=f"wk{i}") for i in range(4)]
            wv = [wpool.tile([128, 512], BF16, name=f"wv{i}", tag=f"wv{i}") for i in range(4)]
            wo = [wpool.tile([128, 512], BF16, name=f"wo{i}", tag=f"wo{i}") for i in range(4)]
            for i in range(4):
                nc.sync.dma_start(wq[i][:], wq_d[i])
                nc.sync.dma_start(wk[i][:], wk_d[i])
                nc.sync.dma_start(wv[i][:], wv_d[i])
                nc.sync.dma_start(wo[i][:], wo_d[i])
            expb = [wpool.tile([128, NH * S], BF16, name=f"expb{t}", tag=f"expb{t}") for t in range(2)]
            for t in range(2):
                nc.sync.dma_start(expb[t][:], expb_d[t])
            qb = wpool.tile([1, 512], BF16, name="qb", tag="qb")
            kb = wpool.tile([1, 512], BF16, name="kb", tag="kb")
            vb = wpool.tile([1, 512], BF16, name="vb", tag="vb")
            ob = wpool.tile([1, 512], BF16, name="ob", tag="ob")
            ones_r = wpool.tile([1, 512], BF16, name="ones_r", tag="ones_r")
            ones_c = wpool.tile([128, 32], BF16, name="ones_c", tag="ones_c")
            nc.sync.dma_start(qb[:], qb_d[:])
            nc.sync.dma_start(kb[:], kb_d[:])
            nc.sync.dma_start(vb[:], vb_d[:])
            nc.sync.dma_start(ob[:], ob_d[:])
            nc.sync.dma_start(ones_r[:], ones_r_d[:])
            nc.sync.dma_start(ones_c[:], ones_c_d[:])
            xT = [xpool.tile([128, TOK], BF16, name=f"xT{i}", tag=f"xT{i}") for i in range(4)]
            for i in range(4):
                nc.sync.dma_start(xT[i][:], xT_d[i])

            for _rep in range(reps):
                # ---- phase 1: QKV projections for all 2048 tokens ----
                qT = [qkpool.tile([128, TOK], BF16, name=f"qT{m}", tag=f"qT{m}") for m in range(4)]
                kT = [qkpool.tile([128, TOK], BF16, name=f"kT{m}", tag=f"kT{m}") for m in range(4)]
                v_sb = [qkpool.tile([128, 512], BF16, name=f"v{s}", tag=f"v{s}")
                        for s in range(TOK // 128)]

                for proj, wt, bt, dst in (("q", wq, qb, qT), ("k", wk, kb, kT)):
                    for m in range(4):
                        for nch in range(4):
                            ps = pa_pool.tile([128, 512], F32, name="pa", tag="pa")
                            sl = slice(nch * 512, (nch + 1) * 512)
                            for kc in range(4):
                                nc.tensor.matmul(
                                    ps[:, :512],
                                    wt[kc][:, m * 128:(m + 1) * 128],
                                    xT[kc][:, sl],
                                    start=(kc == 0), stop=False)
                            nc.tensor.matmul(
                                ps[:, :512],
                                bt[0:1, m * 128:(m + 1) * 128],
                                ones_r[0:1, :512],
                                start=False, stop=True)
                            nc.vector.tensor_copy(dst[m][:, sl], ps[:, :512])

                for sch in range(TOK // 128):
                    ps = pa_pool.tile([128, 512], F32, name="pa", tag="pa")
                    for kc in range(4):
                        nc.tensor.matmul(
                            ps[:, :512],
                            xT[kc][:, sch * 128:(sch + 1) * 128],
                            wv[kc][:, :512],
                            start=(kc == 0), stop=False)
                    nc.tensor.matmul(
                        ps[:, :512], ones_r[0:1, :128], vb[0:1, :512],
                        start=False, stop=True)
                    nc.vector.tensor_copy(v_sb[sch][:], ps[:, :512])

                if debug:
                    dtmp = dpool.tile([128, TOK], F32, name="dtmp", tag="dtmp")
                    nc.vector.tensor_copy(dtmp[:], qT[0][:])
                    nc.sync.dma_start(dbg_qT[:, :], dtmp[:])
                    dtmp2 = dpool.tile([128, TOK], F32, name="dtmp2", tag="dtmp")
                    nc.vector.tensor_copy(dtmp2[:], kT[0][:])
                    nc.sync.dma_start(dbg_kT[:, :], dtmp2[:])
                    dtmp3 = dpool.tile([128, 512], F32, name="dtmp3", tag="dtmp3")
                    nc.vector.tensor_copy(dtmp3[:], v_sb[0][:])
                    nc.sync.dma_start(dbg_v[:, :], dtmp3[:])

                # ---- phase 2: attention per batch ----
                for b in range(BPC):
                    ssl = slice(b * S, (b + 1) * S)
                    E = []
                    for tch in range(2):
                        e0 = epool.tile([128, NH * S], BF16, name="e0", tag="e0")
                        tsl = slice(b * S + tch * 128, b * S + tch * 128 + 128)
                        for hg in range(4):
                            pl = pl_pool.tile([128, 2048], F32, name="pl", tag="pl")
                            for hl in range(4):
                                nc.tensor.matmul(
                                    pl[:, hl * 512:hl * 512 + 256],
                                    kT[hg][32 * hl:32 * hl + 32, tsl],
                                    qT[hg][32 * hl:32 * hl + 32, ssl],
                                    start=True, stop=True,
                                    tile_position=(32 * hl, 0))
                            pl_v = pl.rearrange("p (h x) -> p h x", h=4)[:, :, :256]
                            e0_v = e0[:, hg * 1024:(hg + 1) * 1024].rearrange(
                                "p (h x) -> p h x", h=4)
                            nc.scalar.activation(e0_v, pl_v, AF.Exp)
                        e = epool.tile([128, NH * S], BF16, name="e", tag="e")
                        nc.vector.tensor_tensor(e[:], e0[:], expb[tch][:], OP.mult)
                        E.append(e)
                        if debug and b == 0 and tch == 0:
                            d4 = dpool.tile([128, NH * S], F32, name="d4", tag="d4")
                            nc.vector.tensor_copy(d4[:], e0[:])
                            nc.sync.dma_start(dbg_e0[:, :], d4[:])
                            d5 = dpool.tile([128, NH * S], F32, name="d5", tag="d4")
                            nc.vector.tensor_copy(d5[:], e[:])
                            nc.sync.dma_start(dbg_e[:, :], d5[:])

                    # sums (replicated x32 per head) + reciprocal
                    psum_s = ps_pool.tile([128, 1024], F32, name="ps", tag="ps")
                    for hg in range(4):
                        for j in range(4):
                            n = 4 * hg + j
                            for tch in range(2):
                                nc.tensor.matmul(
                                    psum_s[32 * j:32 * j + 32,
                                           hg * 256:(hg + 1) * 256],
                                    ones_c[:, :32],
                                    E[tch][:, n * 256:(n + 1) * 256],
                                    start=(tch == 0), stop=(tch == 1),
                                    tile_position=(0, 32 * j))
                    r = rpool.tile([128, 1024], F32, name="r", tag="r")
                    if debug and b == 0:
                        d6 = dpool.tile([128, 1024], F32, name="d6", tag="d6")
                        nc.vector.tensor_copy(d6[:], psum_s[:])
                        nc.sync.dma_start(dbg_s[:, :], d6[:])
                    nc.vector.reciprocal_approx_fast(r[:], psum_s[:])
                    if debug and b == 0:
                        nc.sync.dma_start(dbg_r[:, :], r[:])

                    # AV (col-packed) + normalize
                    OT = []
                    for hg in range(4):
                        pa = pa_pool.tile([128, 512], F32, name="pa", tag="pa")
                        for j in range(4):
                            n = 4 * hg + j
                            for tch in range(2):
                                nc.tensor.matmul(
                                    pa[32 * j:32 * j + 32, :256],
                                    v_sb[2 * b + tch][:, n * 32:(n + 1) * 32],
                                    E[tch][:, n * 256:(n + 1) * 256],
                                    start=(tch == 0), stop=(tch == 1),
                                    tile_position=(0, 32 * j))
                        ot = otpool.tile([128, 256], BF16, name="ot", tag="ot")
                        nc.vector.tensor_tensor(
                            ot[:], pa[:, :256], r[:, hg * 256:(hg + 1) * 256],
                            OP.mult)
                        OT.append(ot)
                        if debug and b == 0 and hg == 0:
                            d7 = dpool.tile([128, 256], F32, name="d7", tag="d7")
                            nc.vector.tensor_copy(d7[:], ot[:])
                            nc.sync.dma_start(dbg_ot[:, :], d7[:])

                    # output projection
                    for sch in range(2):
                        po = pa_pool.tile([128, 512], F32, name="pa", tag="pa")
                        for hg in range(4):
                            nc.tensor.matmul(
                                po[:, :512],
                                OT[hg][:, sch * 128:(sch + 1) * 128],
                                wo[hg][:, :512],
                                start=(hg == 0), stop=False)
                        nc.tensor.matmul(
                            po[:, :512], ones_r[0:1, :128], ob[0:1, :512],
                            start=False, stop=True)
                        fs = fpool.tile([128, 512], F32, name="f", tag="f")
                        nc.vector.tensor_copy(fs[:], po[:, :512])
                        nc.sync.dma_start(
                            out_d[b * S + sch * 128: b * S + sch * 128 + 128, :],
                            fs[:])
    nc.compile()
    return nc


def _bias_tables(rel_emb):
    """expb[tch, t_local, n*256+s] = exp(bias[n, s, t]) with t = tch*128+t_local."""
    idx = np.arange(H)
    rel = idx[None, :] - idx[:, None] + (H - 1)          # [a, b] -> b - a + 15
    # bias[n, s, t] = rel_emb[n, th-sh+15, tw-sw+15]; biasT[n, t, s] = bias[n, s, t]
    rh = rel[:, :]                                        # [sh, th]
    biasT = rel_emb[:, rh.T[:, None, :, None], rel.T[None, :, None, :]]
    # biasT[n, th, tw, sh, sw] = rel_emb[n, th-sh+15, tw-sw+15]
    biasT = biasT.reshape(NH, S, S)                       # [n, t, s]
    expb = np.exp(biasT.astype(np.float64)).astype(np.float32)
    expb = np.ascontiguousarray(np.transpose(expb, (1, 0, 2)))  # [t, n, s]
    expb = expb.reshape(2, 128, NH * S)
    return expb.astype(ml_dtypes.bfloat16)


_CACHE = {}


def _get_program(reps=1):
    if reps not in _CACHE:
        _CACHE[reps] = build_program(reps)
    return _CACHE[reps]


def make_in_maps(**inputs):
    x = np.asarray(inputs["x"], np.float32)
    q_w = np.asarray(inputs["q_w"], np.float32).reshape(C, NH * D)
    k_w = np.asarray(inputs["k_w"], np.float32).reshape(C, NH * D)
    v_w = np.asarray(inputs["v_w"], np.float32).reshape(C, NH * D)
    o_w = np.asarray(inputs["o_w"], np.float32).reshape(NH * D, C)
    q_b = np.asarray(inputs["q_b"], np.float32).reshape(NH * D)
    k_b = np.asarray(inputs["k_b"], np.float32).reshape(NH * D)
    v_b = np.asarray(inputs["v_b"], np.float32).reshape(NH * D)
    o_b = np.asarray(inputs["o_b"], np.float32).reshape(C)
    rel_emb = np.asarray(inputs["rel_emb"], np.float32)

    bf = ml_dtypes.bfloat16
    wq = np.ascontiguousarray((q_w * SCALE).reshape(4, 128, 512)).astype(bf)
    wk = np.ascontiguousarray(k_w.reshape(4, 128, 512)).astype(bf)
    wv = np.ascontiguousarray(v_w.reshape(4, 128, 512)).astype(bf)
    wo = np.ascontiguousarray(o_w.reshape(4, 128, 512)).astype(bf)
    expb = _bias_tables(rel_emb)
    qb = (q_b * SCALE).reshape(1, 512).astype(bf)
    kb = k_b.reshape(1, 512).astype(bf)
    vb = v_b.reshape(1, 512).astype(bf)
    ob = o_b.reshape(1, 512).astype(bf)
    ones_r = np.ones((1, 512), bf)
    ones_c = np.ones((128, 32), bf)

    in_maps = []
    for ci in range(NCORES):
        xc = x[ci * BPC:(ci + 1) * BPC].reshape(TOK, C)
        xT = np.ascontiguousarray(xc.T).astype(bf).reshape(4, 128, TOK)
        in_maps.append(dict(
            xT=xT, wq=wq, wk=wk, wv=wv, wo=wo, expb=expb,
            qb=qb, kb=kb, vb=vb, ob=ob, ones_r=ones_r, ones_c=ones_c))
    return in_maps


def kernel(**inputs):
    nc = _get_program(1)
    in_maps = make_in_maps(**inputs)
    res = run_bass_kernel_spmd(nc, in_maps, core_ids=list(range(NCORES)))
    outs = [res.results[ci]["out"].reshape(BPC, S, C) for ci in range(NCORES)]
    return np.concatenate(outs, axis=0).astype(np.float32)
